# revision 41
# baseline (speedup 1.0000x reference)
"""Bass/Trainium2 kernel for nn_DocRelPrompt.

Reference computation (B=64, L=512, H=768, HEAD=64, N_PROMPTS=10, N_LBL=2):
    rel2 = stack([1-r, r], 1)                   # (B, 2)
    hidden_rel = rel2 @ label_prompts           # (B, H)
    Q  = prompts @ ref_qw.T + ref_qb            # (10, HEAD)
    K  = hid @ ref_kw.T + ref_kb                # (B, L, HEAD)
    scores[b,n] = mean_l(Q[n] . K[b,l]) / 8
                = (hsum[b] . (Q@ref_kw)[n] / (512*8)) + (Q[n].ref_kb)/8
    gate = sigmoid(scores)                      # (B, 10)
    doc  = prompts[None] * gate[..., None]      # (B, 10, H)
    out  = concat([doc, hid + hidden_rel[:,None,:]], axis=1)   # (B, 522, H)

(The `_rel_prompts` branch of the reference is computed but unused, so it is
skipped entirely.)

Sharding: pure data-parallel over batch, 8 cores x 8 batches.  The tiny
prompt/weight tensors are folded on the host into W2s (768,10) and c2 (10,)
and replicated; the label-prompt vectors arrive partition-broadcast so the
per-batch hidden_rel row is one DVE op.

Device work per core (8 batches, processed 2 per DMA for ~3MB transfers):
  - DMA 2 batches of hid in as one (128, 2, 4, 768) tile (SP HWDGE ring).
  - ACT: bf16 shadow copy (cheap stationaries / single-pump matmuls).
  - PE: hsum[b] (1, 768) via ones-stationary matmuls, PSUM-accumulated
    over the 4 L-tiles; ACT downcasts to bf16; a tiny SBUF-SBUF DMA drops
    each row into hs_all[b].
  - DVE: rel row = db*r_b + lp0b (scalar_tensor_tensor), then one in-place
    tensor_tensor body = hid + rel (free-dim broadcast); body DMA rides the
    ACT HWDGE ring so in/out transfers overlap.
  - Tail: 6 PE transposes (8,128)->(128,8) build hsumT, 6 bf16 matmuls
    accumulate scores (10, 8), ACT sigmoid(+c2) -> gate, DVE tensor_scalar
    doc rows, one DMA for out[:, :10, :].
"""

import numpy as np

B, L, H, HEAD, NPR, NLBL = 64, 512, 768, 64, 10, 2
NCORES = 8
BLOC = B // NCORES          # 8 batches per core
LT = L // 128               # 4 L-tiles of 128 partitions
HC = H // 128               # 6 H-chunks of 128
B2 = 2                      # batches per DMA iteration
NIT = BLOC // B2

_CACHE = {}


def _build_module():
    from contextlib import ExitStack

    import concourse.bacc as bacc
    import concourse.mybir as mybir
    from concourse.tile import TileContext

    dt = mybir.dt.float32
    bf = mybir.dt.bfloat16
    ADD = mybir.AluOpType.add

    # Bacc (not raw Bass): its compile() legalizes sync waits — TRN2
    # instructions carry at most one wait, extras become event-sem waits.
    nc = bacc.Bacc("TRN2", target_bir_lowering=False, debug=False)
    hid = nc.dram_tensor("hid", [BLOC, L, H], dt, kind="ExternalInput")
    lp0b = nc.dram_tensor("lp0b", [128, H], dt, kind="ExternalInput")
    db = nc.dram_tensor("db", [128, H], dt, kind="ExternalInput")
    rbc = nc.dram_tensor("rbc", [128, BLOC], dt, kind="ExternalInput")
    w2st = nc.dram_tensor("w2st", [128, HC * NPR], bf, kind="ExternalInput")
    id8 = nc.dram_tensor("id8", [BLOC, BLOC], bf, kind="ExternalInput")
    c2 = nc.dram_tensor("c2", [NPR, 1], dt, kind="ExternalInput")
    prom = nc.dram_tensor("prom", [NPR, H], dt, kind="ExternalInput")
    out = nc.dram_tensor("out", [BLOC, NPR + L, H], dt, kind="ExternalOutput")

    hid_r = hid[:].rearrange("b (t p) h -> b p t h", p=128)
    body_r = out[:, NPR:, :].rearrange("b (t p) h -> b p t h", p=128)
    doc_r = out[:, :NPR, :].transpose([1, 0, 2])  # (10, 8, 768)

    with TileContext(nc) as tc, ExitStack() as ctx:
        const = ctx.enter_context(tc.tile_pool(name="const", bufs=1))
        hidp = ctx.enter_context(tc.tile_pool(name="hidp", bufs=4))
        bfp = ctx.enter_context(tc.tile_pool(name="bfp", bufs=3))
        relp = ctx.enter_context(tc.tile_pool(name="relp", bufs=2))
        hsp = ctx.enter_context(tc.tile_pool(name="hsp", bufs=2, space="PSUM"))
        hsbp = ctx.enter_context(tc.tile_pool(name="hsbp", bufs=2))
        sump = ctx.enter_context(tc.tile_pool(name="sump", bufs=1, space="PSUM"))
        scop = ctx.enter_context(tc.tile_pool(name="scop", bufs=1, space="PSUM"))
        warmp = ctx.enter_context(tc.tile_pool(name="warmp", bufs=1, space="PSUM"))
        small = ctx.enter_context(tc.tile_pool(name="small", bufs=1))

        ones_bf = const.tile([128, 1], bf)
        nc.vector.memset(ones_bf[:], 1.0)

        # issue the first two hid loads BEFORE the const loads — the SP
        # HWDGE ring is FIFO, and ~5us of const dispatches would otherwise
        # delay the first big transfer
        t_ins = []
        for b in range(2):
            t_in = hidp.tile([128, LT, H], dt, tag="hid")
            nc.sync.dma_start(t_in[:], hid_r[b])
            t_ins.append(t_in)

        w2st_sb = const.tile([128, HC * NPR], bf)
        nc.sync.dma_start(w2st_sb[:], w2st[:])
        id8_sb = const.tile([BLOC, BLOC], bf)
        nc.sync.dma_start(id8_sb[:], id8[:])
        c2_sb = const.tile([NPR, 1], dt)
        nc.sync.dma_start(c2_sb[:], c2[:])
        prom_sb = const.tile([NPR, H], dt)
        nc.sync.dma_start(prom_sb[:], prom[:])
        lp0b_sb = const.tile([128, H], dt)
        nc.sync.dma_start(lp0b_sb[:], lp0b[:])
        db_sb = const.tile([128, H], dt)
        nc.sync.dma_start(db_sb[:], db[:])
        rbc_sb = const.tile([128, BLOC], dt)
        nc.sync.dma_start(rbc_sb[:], rbc[:])

        # Warm-up matmuls: pre-sync the PE against the DVE memset and the
        # w2st DMA one dependency at a time.
        scrap_ps = warmp.tile([128, 1], dt)
        nc.tensor.matmul(scrap_ps[0:1, :], ones_bf[:], ones_bf[:],
                         start=True, stop=True)
        nc.tensor.matmul(scrap_ps[0:NPR, :], w2st_sb[:, 0:NPR], ones_bf[:],
                         start=True, stop=True)

        hs_all = const.tile([BLOC, H], bf)  # per-batch hsum rows (bf16)

        for b in range(BLOC):
            if b < 2:
                t_in = t_ins[b]
            else:
                t_in = hidp.tile([128, LT, H], dt, tag="hid")
                nc.sync.dma_start(t_in[:], hid_r[b])

            # bf16 shadow for the PE sums (fp32 stationary loads are ~6x
            # slower and fp32 matmuls double-pump; gate path tolerates bf16)
            t_bf = bfp.tile([128, LT, H], bf, tag="hidbf")
            nc.scalar.copy(t_bf[:], t_in[:])

            # hsum[b] (1, 768): ones-stationary matmuls accumulated over
            # L-tiles; 512/256 split keeps outputs inside a PSUM bank
            hs_ps = hsp.tile([1, 1024], dt, tag="hs")
            for t in range(LT):
                nc.tensor.matmul(hs_ps[0:1, 0:512], ones_bf[:],
                                 t_bf[:, t, 0:512],
                                 start=(t == 0), stop=(t == LT - 1))
                nc.tensor.matmul(hs_ps[0:1, 512:H], ones_bf[:],
                                 t_bf[:, t, 512:H],
                                 start=(t == 0), stop=(t == LT - 1))
            hs_bf = hsbp.tile([1, H], bf, tag="hsbf")
            nc.scalar.copy(hs_bf[:], hs_ps[0:1, 0:H])
            # engines cannot shift partitions; a tiny SBUF->SBUF DMA drops
            # the row at partition b for the batched transposes
            nc.sync.dma_start(hs_all[b : b + 1, :], hs_bf[:])

            # rel row = lp0 + r_b * (lp1 - lp0); host pre-broadcast, r_b is
            # a per-partition scalar
            rel_t = relp.tile([128, H], dt, tag="relsb")
            nc.vector.scalar_tensor_tensor(
                rel_t[:], db_sb[:], rbc_sb[:, b : b + 1], lp0b_sb[:],
                mybir.AluOpType.mult, ADD,
            )

            # body = hid + rel (in place; rel broadcast over L-tiles)
            nc.vector.tensor_tensor(
                t_in[:], t_in[:],
                rel_t[:, None, :].broadcast_to([128, LT, H]),
                ADD,
            )
            # out-DMAs ride the ACT HWDGE ring so they don't queue behind the
            # next batch's in-DMA on the SP ring (FIFO per issuing engine)
            nc.scalar.dma_start(body_r[b], t_in[:])

        # tail: transpose hs_all (8, 768) into hsumT chunks (128, 8) and
        # accumulate scores = W2sT.T @ hsumT over the 6 H-chunks
        hsumT_ps = sump.tile([128, HC * BLOC], bf)
        for c in range(HC):
            nc.tensor.transpose(
                hsumT_ps[:, c * BLOC : (c + 1) * BLOC],
                hs_all[:, c * 128 : (c + 1) * 128],
                id8_sb[:],
            )
        hsum_sb = small.tile([128, HC * BLOC], bf)
        nc.scalar.copy(hsum_sb[:], hsumT_ps[:])
        score_ps = scop.tile([NPR, BLOC], dt)
        for c in range(HC):
            nc.tensor.matmul(
                score_ps[:],
                w2st_sb[:, c * NPR : (c + 1) * NPR],
                hsum_sb[:, c * BLOC : (c + 1) * BLOC],
                start=(c == 0),
                stop=(c == HC - 1),
            )

        gate_sb = small.tile([NPR, BLOC], dt)
        nc.scalar.activation(
            gate_sb[:], score_ps[:],
            func=mybir.ActivationFunctionType.Sigmoid,
            bias=c2_sb[:], scale=1.0,
        )

        doc_sb = small.tile([NPR, BLOC, H], dt)
        for b in range(BLOC):
            nc.vector.tensor_scalar(
                doc_sb[:, b, :], prom_sb[:], gate_sb[:, b : b + 1], None,
                mybir.AluOpType.mult,
            )
        nc.sync.dma_start(doc_r, doc_sb[:])

    nc.compile()
    return nc


def _host_fold(relevance, prompts, label_prompts, qw, qb, kw, kb):
    """Fold the tiny projection weights on the host.

    scores[b, n] = hsum[b] . W2s[:, n] + c2[n], with W2s/c2 absorbing the
    1/L mean pooling and the 1/sqrt(HEAD) scaling.
    """
    q = prompts.astype(np.float64) @ qw.astype(np.float64).T + qb.astype(np.float64)
    w2 = q @ kw.astype(np.float64)                               # (10, H)
    w2s = (w2.T / (L * np.sqrt(HEAD))).astype(np.float32)        # (H, 10)
    c2 = ((q @ kb.astype(np.float64)) / np.sqrt(HEAD)).astype(np.float32)  # (10,)
    import ml_dtypes

    # device layout: (128, HC*NPR), free index = c*NPR + n for h = c*128 + p
    w2st = np.ascontiguousarray(
        w2s.reshape(HC, 128, NPR).transpose(1, 0, 2).reshape(128, HC * NPR)
    ).astype(ml_dtypes.bfloat16)
    return w2st, c2.reshape(NPR, 1)


def _prepare_in_maps(
    relevance, hidden_states_src, prompts, label_prompts,
    ref_qw, ref_qb, ref_kw, ref_kb, **_unused,
):
    import ml_dtypes

    relevance = np.asarray(relevance, dtype=np.float32)
    hidden_states_src = np.ascontiguousarray(np.asarray(hidden_states_src, dtype=np.float32))
    prompts = np.ascontiguousarray(np.asarray(prompts, dtype=np.float32))
    label_prompts = np.asarray(label_prompts, dtype=np.float32)

    w2st, c2 = _host_fold(
        relevance, prompts, label_prompts,
        np.asarray(ref_qw, np.float32), np.asarray(ref_qb, np.float32),
        np.asarray(ref_kw, np.float32), np.asarray(ref_kb, np.float32),
    )
    lp0b = np.ascontiguousarray(np.broadcast_to(label_prompts[0], (128, H)))
    dvec = label_prompts[1] - label_prompts[0]
    db = np.ascontiguousarray(np.broadcast_to(dvec, (128, H)))
    id8 = np.eye(BLOC, dtype=ml_dtypes.bfloat16)

    in_maps = []
    for core in range(NCORES):
        sl = slice(core * BLOC, (core + 1) * BLOC)
        in_maps.append(
            {
                "hid": np.ascontiguousarray(hidden_states_src[sl]),
                "lp0b": lp0b,
                "db": db,
                "rbc": np.ascontiguousarray(
                    np.broadcast_to(relevance[sl], (128, BLOC))
                ),
                "w2st": w2st,
                "id8": id8,
                "c2": c2,
                "prom": prompts,
            }
        )
    return in_maps


def _get_module():
    if "nc" not in _CACHE:
        _CACHE["nc"] = _build_module()
    return _CACHE["nc"]


def kernel(**inputs):
    from concourse.bass_utils import run_bass_kernel_spmd

    nc = _get_module()
    in_maps = _prepare_in_maps(**inputs)
    res = run_bass_kernel_spmd(nc, in_maps, list(range(NCORES)))
    return np.concatenate([res.results[c]["out"] for c in range(NCORES)], axis=0)
